# revision 2
# baseline (speedup 1.0000x reference)
"""Trainium2 Bass kernel: rFFT(65536)->keep 4000 bins->LayerNorm(8000)->Linear(8000,512)->SiLU.

Math: 2-level pruned Cooley-Tukey. n = 512*n1 + n2 (n1 in [0,128), n2 in [0,512)).
  k1 = k mod 128, q = k div 128; keep k < 4000 -> q in [0,32), mask (q=31, k1>=32).
  Y[n2,k1]  = sum_n1 x[512*n1+n2] * exp(-2i pi n1 k1 / 128)        (inner DFT, matmul)
  Z[n2,k1]  = Y[n2,k1] * exp(-2i pi n2 k1 / 65536)                 (twiddle, DVE/GpSimd)
  X[q,k1]   = sum_n2 Z[n2,k1] * exp(-2i pi n2 q / 512)             (outer DFT, matmul)
  s[e], e = q*128+k1 (re), 4000+q*128+k1 (im); LayerNorm folded into the linear:
  out = SiLU( (G - mu*c) * istd + d ),  G = s @ A',  A'[e,o] = ln_w[e]*W[o,e],
  c = sum_e ln_w*W, d = ln_b @ W.T + b.
"""

import numpy as np
import ml_dtypes

import concourse.bass as bass
import concourse.tile as tile
from concourse import bacc, mybir
from concourse.bass_utils import run_bass_kernel_spmd

N_CORES = 8
B_FULL = 2048
FFT_N = 65536
N1 = 128      # inner DFT length; k1 = k mod 128
N2 = 512      # outer length; q = k div 128
KEEP = 4000
QK = 32       # q in [0, 32), 32*128 = 4096 bins computed, 96 masked
EPS = 1e-5

f32 = mybir.dt.float32
f32r = mybir.dt.float32r
bf16 = mybir.dt.bfloat16
ALU = mybir.AluOpType
ACT = mybir.ActivationFunctionType
BF16 = ml_dtypes.bfloat16


# ---------------------------------------------------------------- host consts
def _host_consts():
    n1 = np.arange(N1, dtype=np.float64)
    k1 = np.arange(N1, dtype=np.float64)
    n2 = np.arange(N2, dtype=np.float64)
    q = np.arange(QK, dtype=np.float64)

    ang1 = 2.0 * np.pi * np.outer(n1, k1) / N1
    C1, S1 = np.cos(ang1), -np.sin(ang1)
    # pair-packed inner: yv = [Yv_re | Yv_im] accumulates
    #   lhsT=xa with [C1 | S1]  plus  lhsT=xb with [-S1 | C1]
    f1 = np.concatenate([np.concatenate([C1, S1], axis=1),
                         np.concatenate([-S1, C1], axis=1)], axis=0).reshape(
        2, 128, 256)  # f1[0]=for xa, f1[1]=for xb

    angT = 2.0 * np.pi * np.outer(n2, k1) / FFT_N               # [512,128]
    Tc, Ts = np.cos(angT), -np.sin(angT)
    ta = np.zeros((128, 1024)); tb = np.zeros((128, 1024))
    for c in range(4):
        ta[:, c * 256:c * 256 + 128] = Tc[c * 128:(c + 1) * 128]
        ta[:, c * 256 + 128:c * 256 + 256] = Ts[c * 128:(c + 1) * 128]
        tb[:, c * 256:c * 256 + 128] = Ts[c * 128:(c + 1) * 128]
        tb[:, c * 256 + 128:c * 256 + 256] = Tc[c * 128:(c + 1) * 128]

    qall = np.concatenate([np.arange(QK), np.arange(N2 - QK, N2)])  # 64 q values
    angW = 2.0 * np.pi * np.outer(n2, qall.astype(np.float64)) / N2  # [512,64]
    WcT, WsT = np.cos(angW), -np.sin(angW)
    # per chunk c, 3 variants of [re-cols(64) | im-cols(64)] rhs weights:
    #   v0 (for PA_c0): [ Wc |  Ws];  v1 (PA_c1): [-Wc | -Ws];  v2 (PB_c*): [-Ws | Wc]
    wq = np.zeros((128, 4 * 384))
    for c in range(4):
        Wc = WcT[c * 128:(c + 1) * 128]; Ws = WsT[c * 128:(c + 1) * 128]
        base = c * 384
        wq[:, base + 0:base + 64] = Wc
        wq[:, base + 64:base + 128] = Ws
        wq[:, base + 128:base + 192] = -Wc
        wq[:, base + 192:base + 256] = -Ws
        wq[:, base + 256:base + 320] = -Ws
        wq[:, base + 320:base + 384] = Wc

    # separation constants: R (k1 -> 128-k1, zero at dst 0) and S0 (dst 0 <- src 0),
    # scaled +/-0.5, as matmul lhsT [src_k1, dst_k1]
    R = np.zeros((128, 128)); S0 = np.zeros((128, 128))
    for d in range(1, 128):
        R[128 - d, d] = 1.0
    S0[0, 0] = 1.0
    rev = np.concatenate([0.5 * R, 0.5 * S0, -0.5 * R, -0.5 * S0], axis=1)  # [128,512]

    return (f1.astype(np.float32), ta.astype(BF16), tb.astype(BF16),
            wq.astype(BF16), rev.astype(BF16))


def _host_linear(ln_w, ln_b, W, b):
    # A'[e,o] = ln_w[e] * W[o,e]; permuted to my (j,k1) layout with masked tail.
    Af = (ln_w[None, :] * W).T.astype(np.float64)               # [8000, 512]
    Ap = np.zeros((8192, 512))
    for j in range(64):
        for_k1 = np.arange(128)
        if j < 32:
            e = j * 128 + for_k1
            valid = e < KEEP
        else:
            e = KEEP + (j - 32) * 128 + for_k1
            valid = e < 2 * KEEP
        Ap[j * 128 + for_k1[valid]] = Af[e[valid]]
    # SBUF layout ap_w[k1, j*512 + o]
    apw = Ap.reshape(64, 128, 512).transpose(1, 0, 2).reshape(128, 64 * 512)
    cvec = (ln_w[None, :] * W).sum(axis=1)                      # [512]
    dvec = ln_b @ W.T + b                                       # [512]
    cb = np.tile(cvec.astype(np.float32)[None, :], (128, 1))
    db = np.tile(dvec.astype(np.float32)[None, :], (128, 1))
    return apw.astype(BF16), cb, db


# ---------------------------------------------------------------- bass kernel
def build_nc(rows, block, reps=1):
    """Build the per-core Bass program for `rows` batch rows, processed in
    groups of `block` (the LN/linear batch tile, <= 128). reps>1 repeats the
    whole computation back-to-back (for HW timing)."""
    assert rows % block == 0
    nblk = rows // block
    nc = bacc.Bacc("TRN2", target_bir_lowering=False, debug=False)

    xd = nc.dram_tensor("x", [rows, 128, 512], f32r, kind="ExternalInput")
    f1d = nc.dram_tensor("f1", [2, 128, 256], f32r, kind="ExternalInput")
    tad = nc.dram_tensor("ta", [128, 1024], bf16, kind="ExternalInput")
    tbd = nc.dram_tensor("tb", [128, 1024], bf16, kind="ExternalInput")
    wqd = nc.dram_tensor("wq", [128, 1536], bf16, kind="ExternalInput")
    revd = nc.dram_tensor("rev", [128, 512], bf16, kind="ExternalInput")
    apwd = nc.dram_tensor("apw", [128, 64 * 512], bf16, kind="ExternalInput")
    cd = nc.dram_tensor("cvec", [128, 512], f32, kind="ExternalInput")
    dd = nc.dram_tensor("dvec", [128, 512], f32, kind="ExternalInput")
    outd = nc.dram_tensor("out", [nblk, block, 512], f32, kind="ExternalOutput")

    from contextlib import ExitStack
    with tile.TileContext(nc) as tc, ExitStack() as es:
        consts = es.enter_context(tc.tile_pool(name="consts", bufs=1))
        f1a_sb = consts.tile([128, 256], f32r, name="f1a_sb")
        f1b_sb = consts.tile([128, 256], f32r, name="f1b_sb")
        ta_sb = consts.tile([128, 1024], bf16, name="ta_sb")
        tb_sb = consts.tile([128, 1024], bf16, name="tb_sb")
        wq_sb = consts.tile([128, 1536], bf16, name="wq_sb")
        rev_sb = consts.tile([128, 512], bf16, name="rev_sb")
        apw_sb = consts.tile([128, 64 * 512], bf16, name="apw_sb")
        c_sb = consts.tile([128, 512], f32, name="c_sb")
        d_sb = consts.tile([128, 512], f32, name="d_sb")
        ones_sb = consts.tile([128, 1], f32, name="ones_sb")
        # small consts needed by the first pairs go first on the sync queue;
        # the big linear weights (needed ~100us later) go on gpsimd's queue
        nc.sync.dma_start(out=f1a_sb, in_=f1d[0])
        nc.sync.dma_start(out=f1b_sb, in_=f1d[1])
        for sb, dr in ((ta_sb, tad), (tb_sb, tbd), (wq_sb, wqd),
                       (rev_sb, revd)):
            nc.sync.dma_start(out=sb, in_=dr[:])
        for sb, dr in ((apw_sb, apwd), (c_sb, cd), (d_sb, dd)):
            nc.gpsimd.dma_start(out=sb, in_=dr[:])
        nc.vector.memset(ones_sb, 1.0)

        xp = es.enter_context(tc.tile_pool(name="xp", bufs=7))
        yp = es.enter_context(tc.tile_pool(name="yp", bufs=2, space="PSUM"))
        ybp = es.enter_context(tc.tile_pool(name="ybp", bufs=3))
        pp = es.enter_context(tc.tile_pool(name="pp", bufs=3))
        op1 = es.enter_context(tc.tile_pool(name="op1", bufs=2, space="PSUM"))
        pm = es.enter_context(tc.tile_pool(name="pm", bufs=1, space="PSUM"))
        svp = es.enter_context(tc.tile_pool(name="svp", bufs=2))
        sp = es.enter_context(tc.tile_pool(name="sp", bufs=2))
        sqp = es.enter_context(tc.tile_pool(name="sqp", bufs=1))
        stp = es.enter_context(tc.tile_pool(name="stp", bufs=2))
        gp = es.enter_context(tc.tile_pool(name="gp", bufs=1, space="PSUM"))
        smp = es.enter_context(tc.tile_pool(name="smp", bufs=2))
        ep = es.enter_context(tc.tile_pool(name="ep", bufs=1))

        apw3 = apw_sb.rearrange("p (j o) -> p j o", j=64)

        pbk = block // 2  # pairs per block
        import contextlib
        loop_ctx = tc.For_i(0, reps, 1) if reps > 1 else contextlib.nullcontext()
        with loop_ctx:
          for blk in range(nblk):
              sv_buf = svp.tile([128, 128 * pbk], bf16, name="sv_buf")
              sv4 = sv_buf.rearrange("p (jq t) -> p jq t", jq=128)
              s_buf = sp.tile([128, 64 * block], bf16, name="s_buf")
              s3 = s_buf.rearrange("p (j b) -> p j b", j=64)
              s4 = s_buf.rearrange("p (j t u) -> p j t u", j=64, u=2)
              for p in range(pbk):
                  r = blk * block + 2 * p
                  xa_t = xp.tile([128, 512], f32r, name="xa_t")
                  xb_t = xp.tile([128, 512], f32r, name="xb_t")
                  nc.sync.dma_start(out=xa_t, in_=xd[r])
                  nc.sync.dma_start(out=xb_t, in_=xd[r + 1])
                  y_ps = yp.tile([128, 1024], f32, name="y_ps")
                  for c in range(4):
                      reg = y_ps[:, c * 256:(c + 1) * 256]
                      nc.tensor.matmul(reg, lhsT=xa_t[:, c * 128:(c + 1) * 128],
                                       rhs=f1a_sb, start=True, stop=False)
                      nc.tensor.matmul(reg, lhsT=xb_t[:, c * 128:(c + 1) * 128],
                                       rhs=f1b_sb, start=False, stop=True)
                  y_bf = ybp.tile([128, 1024], bf16, name="y_bf")
                  nc.scalar.copy(out=y_bf, in_=y_ps)
                  pa = pp.tile([128, 1024], bf16, name="pa")
                  pb = pp.tile([128, 1024], bf16, name="pb")
                  nc.vector.tensor_mul(pa, y_bf, ta_sb)
                  # split pb by columns: GpSimd is ~3.5x slower per element, so
                  # give it a fixed slice that stays off the critical path
                  nc.gpsimd.tensor_mul(pb[:, 0:512], y_bf[:, 0:512], tb_sb[:, 0:512])
                  nc.vector.tensor_mul(pb[:, 512:1024], y_bf[:, 512:1024],
                                       tb_sb[:, 512:1024])
                  # outer DFT (64 q: 0..31 and 480..511) + twiddle-combines fused
                  # into one 16-matmul accumulation:
                  # o cols = [re-lo 0:32 | re-hi 32:64 | im-lo 64:96 | im-hi 96:128]
                  o = op1.tile([128, 128], f32, name="o")
                  # all pa-quadrant matmuls first: 8x54ns of PE work hides the
                  # slower GpSimd pb-slice (order within one accumulation group
                  # is free after start=True)
                  for c in range(4):
                      w0 = wq_sb[:, c * 384 + 0:c * 384 + 128]
                      w1 = wq_sb[:, c * 384 + 128:c * 384 + 256]
                      nc.tensor.matmul(o, lhsT=pa[:, c * 256:c * 256 + 128],
                                       rhs=w0, start=(c == 0), stop=False)
                      nc.tensor.matmul(o, lhsT=pa[:, c * 256 + 128:c * 256 + 256],
                                       rhs=w1, start=False, stop=False)
                  for ci, c in enumerate((2, 3, 0, 1)):  # DVE-computed pb first
                      w2 = wq_sb[:, c * 384 + 256:c * 384 + 384]
                      nc.tensor.matmul(o, lhsT=pb[:, c * 256:c * 256 + 128],
                                       rhs=w2, start=False, stop=False)
                      nc.tensor.matmul(o, lhsT=pb[:, c * 256 + 128:c * 256 + 256],
                                       rhs=w2, start=False, stop=(ci == 3))
                  nc.scalar.copy(out=sv4[:, 0:64, p:p + 1], in_=o[:, 0:64].unsqueeze(2))
                  nc.vector.tensor_copy(out=sv4[:, 64:128, p:p + 1],
                                        in_=o[:, 64:128].unsqueeze(2))
              # ---- separation: s_a = (Xv[k] + conj(Xv[-k]))/2, s_b = (Xv[k] -
              # conj(Xv[-k]))/(2i), via reversal matmuls (R: k1 -> 128-k1)
              RP, S0P, RN, S0N = (rev_sb[:, 0:128], rev_sb[:, 128:256],
                                  rev_sb[:, 256:384], rev_sb[:, 384:512])
              g_ps = gp.tile([128, 512], f32, name="g_ps")
              for q in range(32):
                  m_re_main = 63 - q
                  m_re_corr = 0 if q == 0 else 64 - q
                  m_im_main = 127 - q
                  m_im_corr = 64 if q == 0 else 128 - q
                  # (out j, out u, self jq, self scale, R lhsT, S0 lhsT, mirror main, corr)
                  specs = [
                      (q, 0, q, 0.5, RP, S0P, m_re_main, m_re_corr),        # a_re
                      (32 + q, 0, 64 + q, 0.5, RN, S0N, m_im_main, m_im_corr),  # a_im
                      (q, 1, 64 + q, 0.5, RP, S0P, m_im_main, m_im_corr),   # b_re
                      (32 + q, 1, q, -0.5, RP, S0P, m_re_main, m_re_corr),  # b_im
                  ]
                  psm = pm.tile([128, 4 * pbk], f32, name="psm", tag="psm")
                  for si, (j, u, selfjq, sc, Rm, S0m, mmain, mcorr) in enumerate(specs):
                      reg = psm[:, si * pbk:(si + 1) * pbk]
                      nc.tensor.matmul(reg, lhsT=Rm, rhs=sv4[:, mmain, :],
                                       start=True, stop=False)
                      nc.tensor.matmul(reg, lhsT=S0m, rhs=sv4[:, mcorr, :],
                                       start=False, stop=True)
                  for si, (j, u, selfjq, sc, Rm, S0m, mmain, mcorr) in enumerate(specs):
                      nc.vector.scalar_tensor_tensor(
                          out=s4[:, j, :, u:u + 1],
                          in0=sv4[:, selfjq, :].unsqueeze(2), scalar=sc,
                          in1=psm[:, si * pbk:(si + 1) * pbk].unsqueeze(2),
                          op0=ALU.mult, op1=ALU.add)
              # mask bins k >= 4000 (q = 31, k1 >= 32), both re and im
              for pbase in (32, 64, 96):
                  nc.vector.memset(s_buf[pbase:pbase + 32, 31 * block:32 * block], 0.0)
                  nc.vector.memset(s_buf[pbase:pbase + 32, 63 * block:64 * block], 0.0)
              for j in range(64):
                  nc.tensor.matmul(
                      g_ps[:block], lhsT=s_buf[:, j * block:(j + 1) * block],
                      rhs=apw3[:, j, :], start=(j == 0), stop=(j == 63))
              # LN stats: per-row sum(s), sum(s^2) over the 8192 slots (masked=0)
              sums_s = stp.tile([128, block], f32, name="sums_s")
              sums_q = stp.tile([128, block], f32, name="sums_q")
              nc.vector.tensor_reduce(
                  out=sums_s, in_=s_buf.rearrange("p (j b) -> p b j", j=64),
                  axis=mybir.AxisListType.X, op=ALU.add)
              sums_qh = stp.tile([128, block], f32, name="sums_qh")
              for half in range(2):
                  sq_buf = sqp.tile([128, 32 * block], bf16, name="sq_buf")
                  nc.scalar.activation(
                      sq_buf, s_buf[:, half * 32 * block:(half + 1) * 32 * block],
                      ACT.Square)
                  nc.vector.tensor_reduce(
                      out=(sums_q if half == 0 else sums_qh),
                      in_=sq_buf.rearrange("p (j b) -> p b j", j=32),
                      axis=mybir.AxisListType.X, op=ALU.add)
              nc.vector.tensor_add(sums_q, sums_q, sums_qh)
              stat_ps = pm.tile([128, pbk], f32, name="stat_ps", tag="psm")
              nc.tensor.matmul(stat_ps[:block, 0:1], lhsT=sums_s,
                               rhs=ones_sb, start=True, stop=True)
              nc.tensor.matmul(stat_ps[:block, 1:2], lhsT=sums_q,
                               rhs=ones_sb, start=True, stop=True)
              mu = smp.tile([128, 1], f32, name="mu")
              negmu = smp.tile([128, 1], f32, name="negmu")
              e2 = smp.tile([128, 1], f32, name="e2")
              varep = smp.tile([128, 1], f32, name="varep")
              rec = smp.tile([128, 1], f32, name="rec")
              istd = smp.tile([128, 1], f32, name="istd")
              nc.vector.tensor_scalar_mul(mu[:block], stat_ps[:block, 0:1], 1.0 / (2 * KEEP))
              nc.vector.tensor_scalar_mul(negmu[:block], stat_ps[:block, 0:1], -1.0 / (2 * KEEP))
              nc.vector.tensor_scalar_mul(e2[:block], stat_ps[:block, 1:2], 1.0 / (2 * KEEP))
              # varep = e2 - mu^2 + EPS = (mu * -mu) + e2, then + EPS
              nc.vector.scalar_tensor_tensor(
                  out=varep[:block], in0=mu[:block], scalar=negmu[:block],
                  in1=e2[:block], op0=ALU.mult, op1=ALU.add)
              nc.vector.tensor_scalar_add(varep[:block], varep[:block], EPS)
              nc.vector.reciprocal(rec[:block], varep[:block])
              nc.scalar.activation(istd[:block], rec[:block], ACT.Sqrt)
              p1 = ep.tile([128, 512], f32, name="p1")
              p2 = ep.tile([128, 512], f32, name="p2")
              o_sb = ep.tile([128, 512], f32, name="o_sb")
              nc.vector.scalar_tensor_tensor(
                  out=p1[:block], in0=c_sb[:block], scalar=negmu[:block],
                  in1=g_ps[:block], op0=ALU.mult, op1=ALU.add)
              nc.vector.scalar_tensor_tensor(
                  out=p2[:block], in0=p1[:block], scalar=istd[:block],
                  in1=d_sb[:block], op0=ALU.mult, op1=ALU.add)
              nc.scalar.activation(o_sb[:block], p2[:block], ACT.Silu)
              nc.sync.dma_start(out=outd[blk], in_=o_sb[:block])

    nc.compile()
    return nc


# ---------------------------------------------------------------- entry points
_CACHE = {}


def _get_nc(rows, block, reps=1):
    key = (rows, block, reps)
    if key not in _CACHE:
        _CACHE[key] = build_nc(rows, block, reps)
    return _CACHE[key]


def make_in_maps(x, ln_w, ln_b, W, b, rows_per_core, n_cores=N_CORES):
    f1, ta, tb, wq, rev = _host_consts()
    apw, cb, db = _host_linear(
        np.asarray(ln_w, np.float64), np.asarray(ln_b, np.float64),
        np.asarray(W, np.float64), np.asarray(b, np.float64))
    x = np.ascontiguousarray(np.asarray(x, np.float32))
    in_maps = []
    for i in range(n_cores):
        xs = x[i * rows_per_core:(i + 1) * rows_per_core].reshape(
            rows_per_core, 128, 512)
        in_maps.append({
            "x": xs, "f1": f1, "ta": ta, "tb": tb, "wq": wq, "rev": rev,
            "apw": apw, "cvec": cb, "dvec": db,
        })
    return in_maps


def run_cores(x, ln_w, ln_b, W, b, rows_per_core, block, n_cores=N_CORES,
              trace=False):
    nc = _get_nc(rows_per_core, block)
    in_maps = make_in_maps(x, ln_w, ln_b, W, b, rows_per_core, n_cores)
    res = run_bass_kernel_spmd(nc, in_maps, core_ids=list(range(n_cores)),
                               trace=trace)
    outs = [res.results[i]["out"].reshape(rows_per_core, 512)
            for i in range(n_cores)]
    return np.concatenate(outs, axis=0), res


def kernel(x, ln_w, ln_b, W, b):
    rows = B_FULL // N_CORES
    out, _ = run_cores(x, ln_w, ln_b, W, b, rows, 128)
    return out.reshape(B_FULL, 1, 512).astype(np.float32)



# revision 7
# speedup vs baseline: 2.0624x; 2.0624x over previous
"""Trainium2 Bass kernel: rFFT(65536)->keep 4000 bins->LayerNorm(8000)->Linear(8000,512)->SiLU.

v2: real-input 2-level Cooley-Tukey (no row pairing).  n = 512*n1 + n2,
k = 128*q + k1.  Per row:
  inner:  Y[n2, j] = sum_n1 x[512 n1 + n2] e^{-2 pi i n1 j/128}, j in [0,64];
          F1 cols = [re j=0..64 | im j=1..63] (Yim0 = 0, Y64 real).
  tw:     pa = y*ta -> [P1=Yre*c | 0 | P2=Yim*s], pb = y*tb -> [P3 | 0 | P4]
          per chunk (col 64 zeroed so P2/P4 j=0 slots read 0).
  outer:  X[qt, j] = sum_n2 Z[n2,j] e^{-2 pi i n2 qt/512}, Z = Y*tw, with
          E-weights as lhsT so out partitions = slots (w*64+m), m<32: qt=m,
          m>=32: qt=448+m.  Mirror bins k=128q+(128-j) = conj X[511-q, j].
  b-path: j=64 bins (k=128q+64) via block-level matmul on Y64 (real).
  LN+linear fold into host weights A''[slot, j, o] (sign+mask baked in);
  sum(s) is a 513th linear column; sum(s^2) via Act-square then 65 masked
  ap=1 matmuls.  Masked/dup slots are zeroed host-side in A''/w1/maskm.
"""

import numpy as np
import ml_dtypes

import concourse.bass as bass
import concourse.tile as tile
from concourse import bacc, mybir
from concourse.bass_utils import run_bass_kernel_spmd

N_CORES = 8
B_FULL = 2048
FFT_N = 65536
KEEP = 4000
EPS = 1e-5

f32 = mybir.dt.float32
bf16 = mybir.dt.bfloat16
ALU = mybir.AluOpType
ACT = mybir.ActivationFunctionType
BF16 = ml_dtypes.bfloat16


# ---------------------------------------------------------------- host consts
def _host_consts():
    n1 = np.arange(128.0)

    F1 = np.zeros((128, 128))
    for t in range(65):
        F1[:, t] = np.cos(2 * np.pi * n1 * t / 128)
    for t in range(65, 128):
        F1[:, t] = -np.sin(2 * np.pi * n1 * (t - 64) / 128)

    ta = np.zeros((128, 512))
    tb = np.zeros((128, 512))
    E = np.zeros((128, 4 * 3 * 128))
    E64 = np.zeros((128, 256))
    qt = np.concatenate([np.arange(32), np.arange(480, 512)]).astype(float)
    for c in range(4):
        n2c = np.arange(c * 128, (c + 1) * 128)[:, None]
        ang = 2 * np.pi * n2c * np.arange(64)[None, :] / FFT_N
        ta[:, c * 128:c * 128 + 64] = np.cos(ang)
        tb[:, c * 128:c * 128 + 64] = -np.sin(ang)
        angh = 2 * np.pi * n2c * np.arange(1, 64)[None, :] / FFT_N
        ta[:, c * 128 + 65:c * 128 + 128] = -np.sin(angh)
        tb[:, c * 128 + 65:c * 128 + 128] = np.cos(angh)

        C = np.cos(2 * np.pi * n2c * qt[None, :] / 512)
        S = -np.sin(2 * np.pi * n2c * qt[None, :] / 512)
        C[:, 32] = 0.0   # qt=480 fully masked (k >= 4000): zero the E col
        S[:, 32] = 0.0
        base = c * 384
        E[:, base + 0:base + 128] = np.concatenate([C, S], axis=1)
        E[:, base + 128:base + 256] = np.concatenate([-C, -S], axis=1)
        E[:, base + 256:base + 384] = np.concatenate([-S, C], axis=1)

        kq = (128 * np.arange(32) + 64)[None, :]
        angb = 2 * np.pi * n2c * kq / FFT_N
        E64[:, c * 64:c * 64 + 32] = np.cos(angb)
        E64[:, c * 64 + 32:c * 64 + 64] = -np.sin(angb)

    return (F1.astype(BF16), ta.astype(BF16), tb.astype(BF16),
            E.astype(BF16), E64.astype(BF16))


def _slot_to_e():
    """(part p, j) -> (e in [0,8000) or -1, sign).  j<=63: p = w*64+m;
    j==64: p = w*32+q for p<64."""
    emap = -np.ones((128, 65), dtype=np.int64)
    smap = np.zeros((128, 65))
    for p in range(128):
        for j in range(65):
            if j == 64:
                if p >= 64:
                    continue
                w, q = divmod(p, 32)
                k = 128 * q + 64
                sign = 1.0
            else:
                w, m = divmod(p, 64)
                if m < 32:
                    k = 128 * m + j
                    sign = 1.0
                else:
                    if j == 0:
                        continue
                    k = 128 * (63 - m + 1) - j
                    sign = -1.0 if w == 1 else 1.0
            if k >= KEEP:
                continue
            emap[p, j] = k + (4000 if w else 0)
            smap[p, j] = sign
    return emap, smap


def _host_linear(ln_w, ln_b, W, b):
    emap, smap = _slot_to_e()
    Af = ln_w[None, :] * W                      # [512, 8000]
    apw = np.zeros((128, 65 * 512))
    w1 = np.zeros((128, 65))
    for j in range(65):
        valid = emap[:, j] >= 0
        e = emap[valid, j]
        apw[valid, j * 512:(j + 1) * 512] = smap[valid, j, None] * Af[:, e].T
        w1[valid, j] = smap[valid, j]
    maskm = (emap >= 0).astype(np.float64)      # [128, 65]
    cvec = Af.sum(axis=1)
    dvec = ln_b @ W.T + b
    cb = np.tile(cvec.astype(np.float32)[None, :], (128, 1))
    db = np.tile(dvec.astype(np.float32)[None, :], (128, 1))
    return apw.astype(BF16), w1.astype(BF16), maskm.astype(BF16), cb, db


# ---------------------------------------------------------------- bass kernel
def build_nc(rows, block, reps=1, sim_safe=False):
    assert rows % block == 0 and block == 128
    nblk = rows // block
    ngrp = rows // 8                 # 8-row DMA groups
    act_out = ACT.Identity if sim_safe else ACT.Silu
    nc = bacc.Bacc("TRN2", target_bir_lowering=False, debug=False)

    xd = nc.dram_tensor("x", [ngrp, 128, 8 * 512], bf16, kind="ExternalInput")
    f1d = nc.dram_tensor("f1", [128, 128], bf16, kind="ExternalInput")
    tad = nc.dram_tensor("ta", [128, 512], bf16, kind="ExternalInput")
    tbd = nc.dram_tensor("tb", [128, 512], bf16, kind="ExternalInput")
    ewd = nc.dram_tensor("ew", [128, 1536], bf16, kind="ExternalInput")
    e64d = nc.dram_tensor("e64", [128, 256], bf16, kind="ExternalInput")
    apwd = nc.dram_tensor("apw", [128, 65 * 512], bf16, kind="ExternalInput")
    w1d = nc.dram_tensor("w1", [128, 65], bf16, kind="ExternalInput")
    mkd = nc.dram_tensor("maskm", [128, 65], bf16, kind="ExternalInput")
    cd = nc.dram_tensor("cvec", [128, 512], f32, kind="ExternalInput")
    dd = nc.dram_tensor("dvec", [128, 512], f32, kind="ExternalInput")
    outd = nc.dram_tensor("out", [nblk, 128, 512], f32, kind="ExternalOutput")

    from contextlib import ExitStack
    import contextlib
    with tile.TileContext(nc) as tc, ExitStack() as es:
        consts = es.enter_context(tc.tile_pool(name="consts", bufs=1))
        f1_sb = consts.tile([128, 128], bf16, name="f1_sb")
        ta_sb = consts.tile([128, 512], bf16, name="ta_sb")
        tb_sb = consts.tile([128, 512], bf16, name="tb_sb")
        ew_sb = consts.tile([128, 1536], bf16, name="ew_sb")
        e64_sb = consts.tile([128, 256], bf16, name="e64_sb")
        mk_sb = consts.tile([128, 65], bf16, name="mk_sb")
        w1_sb = consts.tile([128, 65], bf16, name="w1_sb")
        apw_sb = consts.tile([128, 65 * 512], bf16, name="apw_sb")
        c_sb = consts.tile([128, 512], f32, name="c_sb")
        d_sb = consts.tile([128, 512], f32, name="d_sb")
        for sb, dr in ((f1_sb, f1d), (ta_sb, tad), (tb_sb, tbd),
                       (ew_sb, ewd), (e64_sb, e64d), (mk_sb, mkd),
                       (w1_sb, w1d)):
            nc.sync.dma_start(out=sb, in_=dr[:])
        for sb, dr in ((apw_sb, apwd), (c_sb, cd), (d_sb, dd)):
            nc.gpsimd.dma_start(out=sb, in_=dr[:])

        xp = es.enter_context(tc.tile_pool(name="xp", bufs=3))
        yp = es.enter_context(tc.tile_pool(name="yp", bufs=2, space="PSUM"))
        ysp = es.enter_context(tc.tile_pool(name="ysp", bufs=3))
        pp = es.enter_context(tc.tile_pool(name="pp", bufs=2))
        op = es.enter_context(tc.tile_pool(name="op", bufs=2, space="PSUM"))
        sp = es.enter_context(tc.tile_pool(name="sp", bufs=2))
        sqp = es.enter_context(tc.tile_pool(name="sqp", bufs=1))
        y64p = es.enter_context(tc.tile_pool(name="y64p", bufs=2))
        pm = es.enter_context(tc.tile_pool(name="pm", bufs=1, space="PSUM"))
        pms = es.enter_context(tc.tile_pool(name="pms", bufs=1, space="PSUM"))
        gp = es.enter_context(tc.tile_pool(name="gp", bufs=1, space="PSUM"))
        smp = es.enter_context(tc.tile_pool(name="smp", bufs=2))
        ep = es.enter_context(tc.tile_pool(name="ep", bufs=1))

        loop_ctx = tc.For_i(0, reps, 1) if reps > 1 else contextlib.nullcontext()
        with loop_ctx:
          for blk in range(nblk):
            s_blk = sp.tile([128, 65 * 128], bf16, name="s_blk")
            s3 = s_blk.rearrange("p (j b) -> p j b", j=65)
            y64_blk = y64p.tile([128, 512], bf16, name="y64_blk")
            y64v = y64_blk.rearrange("p (c r) -> p c r", c=4)
            for g in range(16):              # 8 rows = 4 pairs per group
                x_t = xp.tile([128, 8 * 512], bf16, name="x_t")
                nc.sync.dma_start(out=x_t, in_=xd[blk * 16 + g])
                o_ps = op.tile([128, 512], f32, name="o_ps")
                for p in range(4):
                    pa = pp.tile([128, 1024], bf16, name="pa")
                    pb = pp.tile([128, 1024], bf16, name="pb")
                    for r2 in range(2):
                        row = 2 * p + r2
                        y_ps = yp.tile([128, 512], f32, name="y_ps")
                        for c in range(4):
                            nc.tensor.matmul(
                                y_ps[:, c * 128:(c + 1) * 128],
                                lhsT=x_t[:, row * 512 + c * 128:
                                         row * 512 + (c + 1) * 128],
                                rhs=f1_sb, start=True, stop=True)
                        y_sb = ysp.tile([128, 512], bf16, name="y_sb")
                        nc.scalar.copy(out=y_sb, in_=y_ps)
                        ysv = y_sb.rearrange("p (c u) -> p c u", c=4)
                        grow = g * 8 + row
                        nc.gpsimd.tensor_copy(
                            out=y64v[:, :, grow:grow + 1],
                            in_=ysv[:, :, 64:65])
                        nc.vector.tensor_mul(
                            pa[:, r2 * 512:(r2 + 1) * 512], y_sb, ta_sb)
                        nc.vector.tensor_mul(
                            pb[:, r2 * 512:(r2 + 1) * 512], y_sb, tb_sb)
                    pav = pa.rearrange("p (r u) -> p r u", r=2)
                    pbv = pb.rearrange("p (r u) -> p r u", r=2)
                    reg = o_ps[:, p * 128:(p + 1) * 128]
                    nmm = 0
                    for c in range(4):
                        for (src, w, pl) in ((pav, 0, 0), (pav, 1, 1),
                                             (pbv, 0, 2), (pbv, 1, 2)):
                            nc.tensor.matmul(
                                reg,
                                lhsT=ew_sb[:, c * 384 + pl * 128:
                                           c * 384 + (pl + 1) * 128],
                                rhs=src[:, :, c * 128 + w * 64:
                                        c * 128 + (w + 1) * 64],
                                start=(nmm == 0), stop=(nmm == 15))
                            nmm += 1
                ov = o_ps.rearrange("p (pr j) -> p j pr", pr=8)
                nc.vector.tensor_copy(out=s3[:, 0:64, g * 8:(g + 1) * 8],
                                      in_=ov)
            # ---- b-path: j=64 bins from y64_blk
            psb = pm.tile([128, 128], f32, name="psb")
            for c in range(4):
                nc.tensor.matmul(psb[0:64, :],
                                 lhsT=e64_sb[:, c * 64:(c + 1) * 64],
                                 rhs=y64_blk[:, c * 128:(c + 1) * 128],
                                 start=(c == 0), stop=(c == 3))
            nc.vector.tensor_copy(out=s3[0:64, 64, :], in_=psb[0:64, :])
            nc.vector.memset(s3[64:128, 64, :], 0.0)
            # ---- stats: sum s^2 via Act-square + 65 masked ap=1 matmuls
            sq_blk = sqp.tile([128, 65 * 128], bf16, name="sq_blk")
            nc.scalar.activation(sq_blk, s_blk, ACT.Square)
            sq3 = sq_blk.rearrange("p (j b) -> p j b", j=65)
            stat_ps = pms.tile([128, 2], f32, name="stat_ps")
            for j in range(65):
                nc.tensor.matmul(stat_ps[:, 0:1], lhsT=sq3[:, j, :],
                                 rhs=mk_sb[:, j:j + 1],
                                 start=(j == 0), stop=(j == 64))
            for j in range(65):
                nc.tensor.matmul(stat_ps[:, 1:2], lhsT=s3[:, j, :],
                                 rhs=w1_sb[:, j:j + 1],
                                 start=(j == 0), stop=(j == 64))
            # ---- linear (65 matmuls)
            g_ps = gp.tile([128, 512], f32, name="g_ps")
            for j in range(65):
                nc.tensor.matmul(g_ps, lhsT=s3[:, j, :],
                                 rhs=apw_sb[:, j * 512:(j + 1) * 512],
                                 start=(j == 0), stop=(j == 64))
            # ---- LN tail
            mu = smp.tile([128, 1], f32, name="mu")
            negmu = smp.tile([128, 1], f32, name="negmu")
            e2 = smp.tile([128, 1], f32, name="e2")
            varep = smp.tile([128, 1], f32, name="varep")
            rec = smp.tile([128, 1], f32, name="rec")
            istd = smp.tile([128, 1], f32, name="istd")
            nc.vector.tensor_scalar_mul(mu, stat_ps[:, 1:2], 1.0 / (2 * KEEP))
            nc.vector.tensor_scalar_mul(negmu, stat_ps[:, 1:2],
                                        -1.0 / (2 * KEEP))
            nc.vector.tensor_scalar_mul(e2, stat_ps[:, 0:1], 1.0 / (2 * KEEP))
            nc.vector.scalar_tensor_tensor(
                out=varep, in0=mu, scalar=negmu, in1=e2,
                op0=ALU.mult, op1=ALU.add)
            nc.vector.tensor_scalar_add(varep, varep, EPS)
            nc.vector.reciprocal(rec, varep)
            nc.scalar.activation(istd, rec, ACT.Sqrt)
            p1 = ep.tile([128, 512], f32, name="p1")
            p2 = ep.tile([128, 512], f32, name="p2")
            o_sb = ep.tile([128, 512], f32, name="o_sb")
            nc.vector.scalar_tensor_tensor(
                out=p1, in0=c_sb, scalar=negmu, in1=g_ps[:, 0:512],
                op0=ALU.mult, op1=ALU.add)
            nc.vector.scalar_tensor_tensor(
                out=p2, in0=p1, scalar=istd, in1=d_sb,
                op0=ALU.mult, op1=ALU.add)
            nc.scalar.activation(o_sb, p2, act_out)
            nc.sync.dma_start(out=outd[blk], in_=o_sb)

    nc.compile()
    return nc


# ---------------------------------------------------------------- entry points
_CACHE = {}


def _get_nc(rows, block, reps=1, sim_safe=False):
    key = (rows, block, reps, sim_safe)
    if key not in _CACHE:
        _CACHE[key] = build_nc(rows, block, reps, sim_safe)
    return _CACHE[key]


def make_in_maps(x, ln_w, ln_b, W, b, rows_per_core, n_cores=N_CORES):
    f1, ta, tb, ew, e64 = _host_consts()
    apw, w1, maskm, cb, db = _host_linear(
        np.asarray(ln_w, np.float64), np.asarray(ln_b, np.float64),
        np.asarray(W, np.float64), np.asarray(b, np.float64))
    xb = np.asarray(x, np.float32).astype(BF16)
    in_maps = []
    for i in range(n_cores):
        xs = xb[i * rows_per_core:(i + 1) * rows_per_core]
        xs = np.ascontiguousarray(
            xs.reshape(rows_per_core // 8, 8, 128, 512)
            .transpose(0, 2, 1, 3).reshape(rows_per_core // 8, 128, 8 * 512))
        in_maps.append({
            "x": xs, "f1": f1, "ta": ta, "tb": tb, "ew": ew, "e64": e64,
            "apw": apw, "w1": w1, "maskm": maskm, "cvec": cb, "dvec": db,
        })
    return in_maps


def run_cores(x, ln_w, ln_b, W, b, rows_per_core, block, n_cores=N_CORES,
              trace=False):
    nc = _get_nc(rows_per_core, block)
    in_maps = make_in_maps(x, ln_w, ln_b, W, b, rows_per_core, n_cores)
    res = run_bass_kernel_spmd(nc, in_maps, core_ids=list(range(n_cores)),
                               trace=trace)
    outs = [res.results[i]["out"].reshape(rows_per_core, 512)
            for i in range(n_cores)]
    return np.concatenate(outs, axis=0), res


def kernel(x, ln_w, ln_b, W, b):
    rows = B_FULL // N_CORES
    out, _ = run_cores(x, ln_w, ln_b, W, b, rows, 128)
    return out.reshape(B_FULL, 1, 512).astype(np.float32)
